# revision 13
# baseline (speedup 1.0000x reference)
"""GPTQ-style 4-bit quantized linear (x @ dequant(qweight) + bias) on 8 TRN2 cores.

Column-parallel: output dim N=11008 sharded across 8 cores (1376 each,
zero-padded to 1408 = 4 planes x 352). Host prep is bit-layout repacking
only: nibbles repacked along N (8 per int32 word) so one word unpacks to
same-k, different-n values; scales/bias are permuted to the plane-major
column order the unpack naturally produces.

Device kernel per core:
  1. Fused (and, or) tensor_scalar ops turn int32 words into fp16-encoded
     planes in place: value = 1024 + E*q (E in {1,16}), exponent 0x6400.
     5 ops per batch of 4 groups (1 shift + 4 masked ors).
  2. Main matmuls (fp16): lhsT = xT_g [128,16], rhs = plane [128,352],
     4 planes col-tiled concurrently -> per-group partials S_g.
  3. DMA-remap S_g from PSUM [(32j+t), w] to SBUF rhs tiles [g, (t,w)].
  4. Scale matmul (fp32) contracts groups: lhsT = (s/E)-window [32,32];
     the PSUM diagonal is the scaled sum. Extracted by gpsimd
     indirect_copy after an ACT PSUM->SBUF copy.
  5. Correction matmul C^T[n,t] = sum_g SZ[g,n]*xsum[t,g] - bias[n]
     (folds the 1024 offset, GPTQ zeros, bias); subtract; DMA out
     transposed [NPAD, T].

Math: out[t,n] = sum_g s'[g,n]*S_g[t,n] - (sum_g SZ[g,n]*xsum[t,g] - bias[n])
  S_g = sum_{k in g} x_k*(1024 + E q),  s' = s/E,  SZ = s'*zenc + s,
  zenc = fp16-encoded zeros plane (1024 + E z),  xsum[t,g] = sum_{k in g} x_k
  (xsum computed from the same fp16 x as the mains, so the 1024 offset
  cancels exactly).
"""

import numpy as np
from contextlib import ExitStack

import concourse.bass as bass
import concourse.tile as tile
from concourse import mybir, bacc
from concourse.alu_op_type import AluOpType
from concourse.bass_utils import run_bass_kernel_spmd
from concourse.masks import make_identity

MASK_LO = 0x000F000F
MASK_HI = 0x00F000F0
EXP16 = 0x64006400  # fp16 exponent bits: value = 1024 + mantissa-int
N_CORES = 8
GROUPSIZE = 128


class Cfg:
    def __init__(self, K=4096, N_shard=1376, T=16, gbatch=4):
        self.K = K
        self.G = K // GROUPSIZE
        self.T = T
        self.N_shard = N_shard
        per_plane = -(-N_shard // 8) * 2           # fp16 cols per plane, unpadded
        self.PW = -(-per_plane // 32) * 32         # plane width (multiple of 32)
        self.NPAD = 4 * self.PW
        self.NW = self.NPAD // 8                   # int32 words per k-row
        self.R = self.PW // 32                     # 32-wide windows per plane
        self.gbatch = gbatch
        assert self.G % gbatch == 0 and N_shard % 8 == 0


FULL = Cfg()

# ---------------------------------------------------------------- host prep


def _unpack_rows(packed, rows):
    """[rows/8, C] int32, 8 nibbles per word along rows -> [rows, C] uint8."""
    w = packed.view(np.uint32)
    out = np.empty((rows, packed.shape[1]), dtype=np.uint8)
    for b in range(8):
        out[b::8] = ((w >> np.uint32(4 * b)) & np.uint32(0xF)).astype(np.uint8)
    return out


def _unpack_cols(packed):
    """[R, C/8] int32, 8 nibbles per word along cols -> [R, C] uint8."""
    w = packed.view(np.uint32)
    out = np.empty((w.shape[0], w.shape[1] * 8), dtype=np.uint8)
    for b in range(8):
        out[:, b::8] = ((w >> np.uint32(4 * b)) & np.uint32(0xF)).astype(np.uint8)
    return out


def _pack_cols(nib):
    """[R, C] uint8 -> [R, C/8] int32, nibble b of word m = col 8m+b."""
    w = np.zeros((nib.shape[0], nib.shape[1] // 8), dtype=np.uint32)
    for b in range(8):
        w |= nib[:, b::8].astype(np.uint32) << np.uint32(4 * b)
    return w.view(np.int32)


def _perm(cfg):
    """n_perm (plane-major) -> padded-shard orig n. Plane j pos 2m+h <- 8m+j+4h."""
    p = np.empty(cfg.NPAD, dtype=np.int64)
    m = np.arange(cfg.PW // 2)
    for j in range(4):
        for h in range(2):
            p[j * cfg.PW + 2 * m + h] = 8 * m + j + 4 * h
    return p


def _escale(cfg):
    """E per n_perm position: planes 1,3 (HI mask) encode 1024 + 16q."""
    e = np.ones(cfg.NPAD, dtype=np.float32)
    e[cfg.PW:2 * cfg.PW] = 16.0
    e[3 * cfg.PW:] = 16.0
    return e


def host_prep(cfg, x, qweight, qzeros, scales, bias):
    nib = _unpack_rows(np.asarray(qweight), cfg.K)
    znib = _unpack_cols(np.asarray(qzeros))
    perm, e = _perm(cfg), _escale(cfg)
    in_maps = []
    for c in range(N_CORES):
        sl = slice(c * cfg.N_shard, (c + 1) * cfg.N_shard)
        nib_s = np.zeros((cfg.K, cfg.NPAD), dtype=np.uint8)
        nib_s[:, : cfg.N_shard] = nib[:, sl]
        znib_s = np.zeros((cfg.G, cfg.NPAD), dtype=np.uint8)
        znib_s[:, : cfg.N_shard] = znib[:, sl]
        s_s = np.zeros((cfg.G, cfg.NPAD), dtype=np.float32)
        s_s[:, : cfg.N_shard] = scales[:, sl]
        b_s = np.zeros(cfg.NPAD, dtype=np.float32)
        b_s[: cfg.N_shard] = bias[sl]
        s_p = s_s[:, perm]
        spv = (s_p / e[None, :]).astype(np.float16)
        in_maps.append(
            {
                "qw": _pack_cols(nib_s),
                "qz": _pack_cols(znib_s),
                "sp": spv,
                "s2": (s_p - 1024.0 * spv.astype(np.float64)).astype(np.float32),
                "biasp": (-b_s[perm]).astype(np.float32),
                "x": np.asarray(x, dtype=np.float32),
            }
        )
    return in_maps


def host_gather(cfg, results):
    perm = _perm(cfg)
    valid = perm < cfg.N_shard
    out = np.empty((cfg.T, cfg.N_shard * N_CORES), dtype=np.float32)
    for c in range(N_CORES):
        oT = results[c]["outT"]  # [NPAD, T]
        shard = np.empty((cfg.T, cfg.N_shard), dtype=np.float32)
        shard[:, perm[valid]] = oT[valid].T
        out[:, c * cfg.N_shard:(c + 1) * cfg.N_shard] = shard
    return out


# ---------------------------------------------------------------- device kernel


def build_kernel(nc, cfg):
    f32, f16, i32 = mybir.dt.float32, mybir.dt.float16, mybir.dt.int32
    K, G, T, PW, NW, R, GB = cfg.K, cfg.G, cfg.T, cfg.PW, cfg.NW, cfg.R, cfg.gbatch
    NPAD = cfg.NPAD
    NI = NPAD // 2  # int32 view columns of one group's encoded planes

    qw_d = nc.declare_dram_parameter("qw", [K, NW], i32, isOutput=False)
    qz_d = nc.declare_dram_parameter("qz", [G, NW], i32, isOutput=False)
    sp_d = nc.declare_dram_parameter("sp", [G, NPAD], f16, isOutput=False)
    s2_d = nc.declare_dram_parameter("s2", [G, NPAD], f32, isOutput=False)
    bias_d = nc.declare_dram_parameter("biasp", [NPAD], f32, isOutput=False)
    x_d = nc.declare_dram_parameter("x", [T, K], f32, isOutput=False)
    out_d = nc.declare_dram_parameter("outT", [NPAD, T], f32, isOutput=True)
    scr_d = nc.dram_tensor("scratch", [R, 128, 512], f32).ap()

    with tile.TileContext(nc) as tc, ExitStack() as ctx:
        singles = ctx.enter_context(tc.tile_pool(name="singles", bufs=1))
        qwp = ctx.enter_context(tc.tile_pool(name="qwp", bufs=2))
        encp = ctx.enter_context(tc.tile_pool(name="encp", bufs=2))
        smallp = ctx.enter_context(tc.tile_pool(name="smallp", bufs=2))
        ps_main = ctx.enter_context(tc.tile_pool(name="ps_main", bufs=1, space="PSUM"))
        ps_sc = ctx.enter_context(tc.tile_pool(name="ps_sc", bufs=2, space="PSUM"))
        ps_c = ctx.enter_context(tc.tile_pool(name="ps_c", bufs=2, space="PSUM"))

        # ---------- phase 0: x prep ----------
        xs = singles.tile([T, K], f32)
        nc.sync.dma_start(out=xs[:], in_=x_d[:])
        xh = singles.tile([T, K], f16)
        nc.vector.tensor_copy(xh[:], xs[:])
        # token dim padded to 32 so col-tiled matmuls write full 32-row
        # PSUM strips (CoreSim rejects partially-written PSUM reads)
        xT = singles.tile([128, G * 32], f16)
        nc.vector.memset(xT[:], 0.0)
        for g in range(G):
            nc.sync.dma_start_transpose(
                out=xT[:, g * 32:g * 32 + T], in_=xh[:, g * 128:(g + 1) * 128]
            )
        ones16 = singles.tile([128, 1], f16)
        nc.vector.memset(ones16[:], 1.0)

        xsumP = ps_sc.tile([T, G], f32, tag="sc")
        for g in range(G):
            nc.tensor.matmul(
                xsumP[:, g:g + 1], xT[:, g * 32:g * 32 + T], ones16[:],
                start=True, stop=True,
            )
        xsum_s = singles.tile([T, G], f32)
        nc.scalar.copy(xsum_s[:], xsumP[:])
        ident = singles.tile([T, T], f32)
        make_identity(nc, ident[:])
        xsT_P = ps_sc.tile([G, T], f32, tag="sc")
        nc.tensor.transpose(xsT_P[:], xsum_s[:], ident[:])
        xsum_aug = singles.tile([G + 1, T], f32)
        nc.vector.memset(xsum_aug[:], 1.0)
        nc.scalar.copy(xsum_aug[:G, :], xsT_P[:])
        # offv[32j+t, g] = -1024 * xsum[t, g]; rows 16-31 of each strip unused
        offv = singles.tile([128, G], f32)
        nc.vector.memset(offv[:], 0.0)
        for j in range(4):
            nc.scalar.mul(offv[32 * j:32 * j + T, :], xsumP[:], -1024.0)

        # ---------- phase 1: scales / zeros prep ----------
        sp16 = singles.tile([G, NPAD], f16)
        nc.sync.dma_start(out=sp16[:], in_=sp_d[:])
        sp_s = singles.tile([G, NPAD], f32)
        nc.vector.tensor_copy(sp_s[:], sp16[:])
        s_s = singles.tile([G, NPAD], f32)
        nc.sync.dma_start(out=s_s[:], in_=s2_d[:])

        qz_t = singles.tile([G, NW], i32)
        nc.sync.dma_start(out=qz_t[:], in_=qz_d[:])
        zs = singles.tile([G, NW], i32)
        nc.vector.memset(zs[:, NW - 1:NW], 0)
        nc.sync.dma_start(
            out=zs[:].bitcast(mybir.dt.uint8)[:, 0:4 * NW - 1],
            in_=qz_t[:].bitcast(mybir.dt.uint8)[:, 1:4 * NW],
        )
        zenc = singles.tile([G, NPAD], f16)
        zi = zenc[:].bitcast(i32)
        for j, (src, mask) in enumerate(
            [(qz_t, MASK_LO), (qz_t, MASK_HI), (zs, MASK_LO), (zs, MASK_HI)]
        ):
            nc.vector.tensor_scalar(
                out=zi[:, j * NW:(j + 1) * NW], in0=src[:],
                scalar1=mask, scalar2=EXP16,
                op0=AluOpType.bitwise_and, op1=AluOpType.bitwise_or,
            )
        zf = singles.tile([G, NPAD], f32)
        nc.vector.tensor_copy(zf[:], zenc[:])
        szb = singles.tile([G + 1, NPAD], f32)
        nc.vector.tensor_tensor(szb[:G, :], zf[:], sp_s[:], AluOpType.mult)
        nc.vector.tensor_tensor(szb[:G, :], szb[:G, :], s_s[:], AluOpType.add)
        nc.sync.dma_start(out=szb[G:G + 1, :], in_=bias_d[None, :])

        # ---------- phase 2: unpack + mains + evac ----------
        rhsbig = []
        for j in range(4):
            rb = singles.tile([G, T * PW], f16, tag=f"rhs{j}", name=f"rhsbig{j}")
            rhsbig.append(rb)
        scopy = singles.tile([128, G * PW], f16)
        for rd in range(G // GB):
            wt = qwp.tile([128, GB * NW], i32, tag="wt")
            for gg in range(GB):
                g = rd * GB + gg
                nc.sync.dma_start(
                    out=wt[:, gg * NW:(gg + 1) * NW],
                    in_=qw_d[g * 128:(g + 1) * 128, :],
                )
            # ws = wt >> 8, via a 1-byte-offset DMA copy (GPSIMD has no
            # tensor_scalar on HW). The stray top byte per word is masked out.
            ws = qwp.tile([128, GB * NW], i32, tag="ws")
            nc.vector.memset(ws[:, GB * NW - 1:GB * NW], 0)
            nc.sync.dma_start(
                out=ws[:].bitcast(mybir.dt.uint8)[:, 0:4 * GB * NW - 1],
                in_=wt[:].bitcast(mybir.dt.uint8)[:, 1:4 * GB * NW],
            )
            enc = encp.tile([128, GB * NPAD], f16, tag="enc")
            ei = enc[:].bitcast(i32)
            specs = [
                (wt, MASK_LO, 0, nc.vector),
                (wt, MASK_HI, 1, nc.vector),
                (ws, MASK_LO, 2, nc.vector),
                (ws, MASK_HI, 3, nc.vector),
            ]
            for src, mask, j, eng in specs:
                for gg in range(GB):
                    eng.tensor_scalar(
                        out=ei[:, gg * NI + j * NW: gg * NI + (j + 1) * NW],
                        in0=src[:, gg * NW:(gg + 1) * NW],
                        scalar1=mask, scalar2=EXP16,
                        op0=AluOpType.bitwise_and, op1=AluOpType.bitwise_or,
                    )
            for gg in range(GB):
                g = rd * GB + gg
                mainP = ps_main.tile([128, 512], f32, tag=f"m{g % 4}", name=f"mainP{g % 4}")
                for j in range(4):
                    nc.tensor.matmul(
                        mainP[32 * j:32 * (j + 1), 0:PW],
                        xT[:, g * 32:(g + 1) * 32],
                        enc[:, gg * NPAD + j * PW: gg * NPAD + (j + 1) * PW],
                        start=True, stop=True, tile_position=(0, 32 * j),
                    )
                # evacuate PSUM -> SBUF (compute engines; DMA can't read PSUM),
                # alternating DVE/ACT to split the cost
                if g % 2 == 0:
                    nc.vector.tensor_scalar(
                        out=scopy[:, g * PW:(g + 1) * PW], in0=mainP[:, 0:PW],
                        scalar1=offv[:, g:g + 1], scalar2=None,
                        op0=AluOpType.add,
                    )
                else:
                    nc.scalar.activation(
                        scopy[:, g * PW:(g + 1) * PW], mainP[:, 0:PW],
                        mybir.ActivationFunctionType.Identity,
                        bias=offv[:, g:g + 1], scale=1.0,
                    )
                # partition-remap into group-major rhs tiles (SBUF->SBUF DMA)
                for j in range(4):
                    nc.sync.dma_start(
                        out=rhsbig[j][g:g + 1, :].rearrange(
                            "o (t w) -> o t w", t=T
                        ),
                        in_=scopy[32 * j:32 * j + T, g * PW:(g + 1) * PW],
                    )

        # ---------- phase 3: scale matmul + correction + diag out ----------
        for r in range(R):
            scP = ps_sc.tile([128, 512], f32, tag="sc", name=f"scP{r}")
            cP = ps_c.tile([128, T], f32, tag="c", name=f"cP{r}")
            for j in range(4):
                # rhs free order (w, t): diagonal lands in contiguous 16-runs
                rhs_ap = rhsbig[j][:, :].rearrange("g (t w) -> g w t", t=T)[
                    :, 32 * r:32 * r + 32, :
                ]
                nc.tensor.matmul(
                    scP[32 * j:32 * (j + 1), :],
                    sp16[:, j * PW + 32 * r: j * PW + 32 * r + 32],
                    rhs_ap,
                    start=True, stop=True, tile_position=(0, 32 * j),
                )
                nc.tensor.matmul(
                    cP[32 * j:32 * (j + 1), :],
                    szb[:, j * PW + 32 * r: j * PW + 32 * r + 32],
                    xsum_aug[:],
                    start=True, stop=True, tile_position=(0, 32 * j),
                )
            scS = smallp.tile([128, 512], f32, tag="scS")
            if r % 2 == 0:
                nc.vector.tensor_copy(scS[:], scP[:])
            else:
                nc.scalar.copy(scS[:], scP[:])
            nc.sync.dma_start(out=scr_d[r], in_=scS[:])
            # diag gather from DRAM: scratch[r][32j+a, a*16+t]
            diagbuf = smallp.tile([128, T], f32, tag="diagbuf")
            diag_src = bass.AP(
                tensor=scr_d.tensor,
                offset=scr_d.offset + r * 128 * 512,
                ap=[[32 * 512, 4], [512 + 16, 32], [1, T]],
            )
            nc.sync.dma_start(out=diagbuf[:], in_=diag_src)
            oT = smallp.tile([128, T], f32, tag="oT")
            nc.vector.scalar_tensor_tensor(
                out=oT[:], in0=diagbuf[:], scalar=0.0, in1=cP[:],
                op0=AluOpType.bypass, op1=AluOpType.subtract,
            )
            nc.sync.dma_start(
                out=out_d[:].rearrange("(j w) t -> j w t", j=4)[
                    :, 32 * r:32 * r + 32, :
                ],
                in_=oT[:],
            )
    return nc


# ---------------------------------------------------------------- entry

_CACHE = {}


def _get_nc(cfg):
    key = (cfg.K, cfg.NPAD, cfg.T)
    if key not in _CACHE:
        nc = bacc.Bacc(num_devices=N_CORES)
        build_kernel(nc, cfg)
        nc.compile()
        _CACHE[key] = nc
    return _CACHE[key]


def kernel(x, qweight, qzeros, scales, bias):
    cfg = FULL
    in_maps = host_prep(cfg, x, qweight, qzeros, scales, bias)
    nc = _get_nc(cfg)
    res = run_bass_kernel_spmd(nc, in_maps, core_ids=list(range(N_CORES)))
    return host_gather(cfg, res.results)
